# revision 3
# baseline (speedup 1.0000x reference)
"""GRU-variant Bass kernel for Trainium2, data-parallel over batch on 8 cores.

Math (per step t, per batch row):
    cat = [x_t, h]                       # [B, 768]
    z   = sigmoid(cat @ Wz.T)            # [B, 512]
    r   = sigmoid(cat @ Wr.T)            # [B, 768]
    ht  = tanh((r * cat) @ Wh.T)         # [B, 512]
    h   = (1-z)*h + z*ht

Strategy (v3):
  - batch 64 split 8 ways -> 8 rows per core, weights replicated.
  - Transposed on-chip layout: features on partitions, batch on free axis.
  - r-gate x-projections are pre-accumulated ONCE PER CHUNK directly into
    PSUM: tile [128, 6*512] spans 6 banks, one bank per output m-tile, so
    each bank holds exactly one accumulation group for the whole chunk
    (PSUM allows a group to pause while other banks run groups, but not
    two interleaved groups in one bank). Per-step h-matmuls accumulate on
    top (start=False); sigmoid reads PSUM directly. The z-gate and g
    matmuls use per-step contiguous groups in their own banks (x-part
    matmuls ride in the z burst).
  - Per-step cat buffer [x|h] in SBUF: single fused r*cat multiply, the
    h-update writes straight into the next step's cat slot, and the
    output DMA reads the h slots (no copies).
  - Tail trick: zc=1-z and m1=zc*h computed off the critical path; after
    tanh only m2=z*g and h'=m1+m2 remain.
  - Output stored bf16, converted to fp32 on host.
"""

import sys

sys.path.insert(0, "/opt/trn_rl_repo")

import numpy as np
import ml_dtypes

import concourse.bass as bass
import concourse.bacc as bacc
import concourse.mybir as mybir
from concourse.bass import ds
from concourse.tile import TileContext
from concourse.bass_utils import run_bass_kernel_spmd

BF16 = ml_dtypes.bfloat16

L, B, D, LAT = 2048, 64, 256, 512
CAT = D + LAT  # 768
NCORES = 8
BL = B // NCORES  # 8 local batch rows
CH = 64  # timesteps per chunk
FP32 = mybir.dt.float32
BF = mybir.dt.bfloat16
AF = mybir.ActivationFunctionType
ALU = mybir.AluOpType


def build_gru_nc(length=L, ch=CH):
    nc = bacc.Bacc("TRN2", target_bir_lowering=False)

    # ---- DRAM I/O ----
    xt = nc.dram_tensor("xt", [D, length, BL], BF, kind="ExternalInput")
    w_zx = nc.dram_tensor("w_zx", [128, 2 * LAT], BF, kind="ExternalInput")
    w_zh = nc.dram_tensor("w_zh", [128, 4 * LAT], BF, kind="ExternalInput")
    w_rx = nc.dram_tensor("w_rx", [128, 2 * CAT], BF, kind="ExternalInput")
    w_rh = nc.dram_tensor("w_rh", [128, 4 * CAT], BF, kind="ExternalInput")
    w_hx = nc.dram_tensor("w_hx", [128, 2 * LAT], BF, kind="ExternalInput")
    w_hh = nc.dram_tensor("w_hh", [128, 4 * LAT], BF, kind="ExternalInput")
    hs = nc.dram_tensor("hs", [LAT, length, BL], BF, kind="ExternalOutput")

    with TileContext(nc) as tc:
        with (
            tc.tile_pool(name="wpool", bufs=1) as wpool,
            tc.tile_pool(name="sbuf", bufs=1) as sb,
            tc.tile_pool(name="psum", bufs=1, space="PSUM") as pp,
        ):
            # weights resident in SBUF
            s_zx = wpool.tile([128, 2 * LAT], BF, tag="zx")
            s_zh = wpool.tile([128, 4 * LAT], BF, tag="zh")
            s_rx = wpool.tile([128, 2 * CAT], BF, tag="rx")
            s_rh = wpool.tile([128, 4 * CAT], BF, tag="rh")
            s_hx = wpool.tile([128, 2 * LAT], BF, tag="hx")
            s_hh = wpool.tile([128, 4 * LAT], BF, tag="hh")
            for dst, src in [
                (s_zx, w_zx), (s_zh, w_zh), (s_rx, w_rx),
                (s_rh, w_rh), (s_hx, w_hx), (s_hh, w_hh),
            ]:
                nc.sync.dma_start(dst[:, :], src[:, :])

            # cat chunk buffer: [p, slot(ch+1), j(6), b(8)] bf16
            #   j<2: x k-tiles; j>=2: h k-tiles. slot s holds cat for step s;
            #   step s writes h' into slot s+1.
            catc = sb.tile([128, (ch + 1) * 48], BF, tag="catc")
            catv = catc[:, :].rearrange("p (s j b) -> p s j b", j=6, b=BL)

            # step temporaries
            rb = sb.tile([128, 48], BF, tag="rb")
            rcb = sb.tile([128, 48], BF, tag="rcb")
            zb = sb.tile([128, 32], BF, tag="zb")
            zcb = sb.tile([128, 32], BF, tag="zcb")
            gt = sb.tile([128, 32], BF, tag="gt")
            m1 = sb.tile([128, 32], BF, tag="m1")
            m2 = sb.tile([128, 32], BF, tag="m2")
            rbv = rb[:, :].rearrange("p (j b) -> p j b", b=BL)
            rcbv = rcb[:, :].rearrange("p (j b) -> p j b", b=BL)
            zbv = zb[:, :].rearrange("p (m b) -> p m b", b=BL)
            zcbv = zcb[:, :].rearrange("p (m b) -> p m b", b=BL)
            gtv = gt[:, :].rearrange("p (m b) -> p m b", b=BL)
            m1v = m1[:, :].rearrange("p (m b) -> p m b", b=BL)
            m2v = m2[:, :].rearrange("p (m b) -> p m b", b=BL)

            # r-gate PSUM: 6 banks, one per m-tile; whole chunk per bank.
            # [p, m(6), s(ch), b(8)] ; col = m*512 + s*8 + b
            pr = pp.tile([128, 6 * ch * BL], FP32, tag="pr")
            prv = pr[:, :].rearrange("p (m s b) -> p m s b", s=ch, b=BL)
            # z / g PSUM: per-step groups, ping-pong by step parity.
            pz = pp.tile([128, 2 * 4 * BL], FP32, tag="pz")
            pzv = pz[:, :].rearrange("p (h m b) -> p h m b", m=4, b=BL)
            pg = pp.tile([128, 2 * 4 * BL], FP32, tag="pg")
            pgv = pg[:, :].rearrange("p (h m b) -> p h m b", m=4, b=BL)

            # initial h = 0 (carry slot)
            nc.vector.memset(catv[:, ch, 2:6, :], 0.0)

            with tc.For_i(
                0, length, ch,
                staggered_reset=True,
                hint_engines=(
                    mybir.EngineType.PE,
                    mybir.EngineType.DVE,
                    mybir.EngineType.Activation,
                    mybir.EngineType.SP,
                ),
            ) as i0:
                # ---- load x chunk (transposed: d on partitions) ----
                for k in range(2):
                    nc.sync.dma_start(
                        catv[:, 0:ch, k, :],
                        xt[128 * k : 128 * (k + 1), ds(i0, ch), :],
                    )
                # carry h from previous chunk (slot ch -> slot 0)
                nc.vector.tensor_copy(catv[:, 0, 2:6, :], catv[:, ch, 2:6, :])

                # r x-projections for the whole chunk (open one group per bank)
                for m in range(6):
                    for k in range(2):
                        nc.tensor.matmul(
                            prv[:, m, :, :],
                            s_rx[:, k * CAT + m * 128 : k * CAT + (m + 1) * 128],
                            catv[:, 0:ch, k, :],
                            start=(k == 0),
                            stop=False,
                            skip_group_check=True,
                        )

                for s in range(ch):
                    half = s % 2

                    # ---- r h-matmuls accumulate on top of x-proj ----
                    for m in range(6):
                        for k in range(4):
                            nc.tensor.matmul(
                                prv[:, m, s, :],
                                s_rh[:, k * CAT + m * 128 : k * CAT + (m + 1) * 128],
                                catv[:, s, 2 + k, :],
                                start=False,
                                stop=(k == 3),
                                skip_group_check=True,
                            )
                    # ---- z: full per-step groups (x + h contraction) ----
                    for m in range(4):
                        for k in range(2):
                            nc.tensor.matmul(
                                pzv[:, half, m, :],
                                s_zx[:, k * LAT + m * 128 : k * LAT + (m + 1) * 128],
                                catv[:, s, k, :],
                                start=(k == 0),
                                stop=False,
                            )
                        for k in range(4):
                            nc.tensor.matmul(
                                pzv[:, half, m, :],
                                s_zh[:, k * LAT + m * 128 : k * LAT + (m + 1) * 128],
                                catv[:, s, 2 + k, :],
                                start=False,
                                stop=(k == 3),
                            )

                    # ---- gates ----
                    nc.scalar.activation(rbv, prv[:, :, s, :], AF.Sigmoid)
                    nc.scalar.activation(zbv, pzv[:, half, :, :], AF.Sigmoid)
                    # rc = r * cat  (critical); zc = 1 - z; m1 = zc * h
                    nc.vector.tensor_mul(rcbv, rbv, catv[:, s, :, :])
                    nc.vector.tensor_scalar(
                        zcb[:, :], zb[:, :], -1.0, 1.0, ALU.mult, ALU.add
                    )
                    nc.vector.tensor_mul(m1v, zcbv, catv[:, s, 2:6, :])

                    # ---- g matmuls ----
                    for m in range(4):
                        for j in range(6):
                            if j < 2:
                                w = s_hx[:, j * LAT + m * 128 : j * LAT + (m + 1) * 128]
                            else:
                                w = s_hh[
                                    :, (j - 2) * LAT + m * 128 : (j - 2) * LAT + (m + 1) * 128
                                ]
                            nc.tensor.matmul(
                                pgv[:, half, m, :],
                                w,
                                rcbv[:, j, :],
                                start=(j == 0),
                                stop=(j == 5),
                            )

                    # ---- tail: h' = m1 + z*g ----
                    nc.scalar.activation(gtv, pgv[:, half, :, :], AF.Tanh)
                    nc.vector.tensor_mul(m2v, zbv, gtv)
                    nc.vector.tensor_add(catv[:, s + 1, 2:6, :], m1v, m2v)

                # ---- store chunk output (h slots 1..ch) ----
                for k in range(4):
                    nc.sync.dma_start(
                        hs[128 * k : 128 * (k + 1), ds(i0, ch), :],
                        catv[:, 1 : ch + 1, 2 + k, :],
                    )
    nc.compile()
    return nc


def _pack_lhsT(w):
    """[K, M] lhsT -> [128, (K//128)*M] packed, col = ktile*M + m."""
    K, M = w.shape
    return (
        w.reshape(K // 128, 128, M).transpose(1, 0, 2).reshape(128, -1)
    )


def prep_weights(Wz, Wr, Wh):
    out = {}
    for name, W, xd in [("z", Wz, LAT), ("r", Wr, CAT), ("h", Wh, LAT)]:
        lhsT_x = _pack_lhsT(np.ascontiguousarray(W[:, :D].T))  # [256, M]
        lhsT_h = _pack_lhsT(np.ascontiguousarray(W[:, D:].T))  # [512, M]
        out[f"w_{name}x"] = lhsT_x.astype(BF16)
        out[f"w_{name}h"] = lhsT_h.astype(BF16)
    return out


_nc_cache = {}


def kernel(x, Wz, Wr, Wh, _nc_cache=_nc_cache):
    x = np.asarray(x, np.float32)
    Wz = np.asarray(Wz, np.float32)
    Wr = np.asarray(Wr, np.float32)
    Wh = np.asarray(Wh, np.float32)

    key = "nc"
    if key not in _nc_cache:
        _nc_cache[key] = build_gru_nc()
    nc = _nc_cache[key]

    wmap = prep_weights(Wz, Wr, Wh)
    xt_all = np.ascontiguousarray(x.transpose(2, 0, 1)).astype(BF16)  # [D, L, B]

    in_maps = []
    for c in range(NCORES):
        m = dict(wmap)
        m["xt"] = np.ascontiguousarray(xt_all[:, :, c * BL : (c + 1) * BL])
        in_maps.append(m)

    res = run_bass_kernel_spmd(nc, in_maps, core_ids=list(range(NCORES)))
    outs = []
    for c in range(NCORES):
        hsT = np.asarray(res.results[c]["hs"]).astype(np.float32)  # [LAT, L, BL]
        outs.append(hsT.transpose(1, 2, 0))  # [L, BL, LAT]
    return np.concatenate(outs, axis=1)  # [L, B, LAT]


# revision 4
# speedup vs baseline: 1.9282x; 1.9282x over previous
"""GRU-variant Bass kernel for Trainium2, data-parallel over batch on 8 cores.

Math (per step t, per batch row):
    cat = [x_t, h]                       # [B, 768]
    z   = sigmoid(cat @ Wz.T)            # [B, 512]
    r   = sigmoid(cat @ Wr.T)            # [B, 768]
    ht  = tanh((r * cat) @ Wh.T)         # [B, 512]
    h   = (1-z)*h + z*ht

Strategy (v4):
  - batch 64 split 8 ways -> 8 rows per core, weights replicated.
  - Transposed on-chip layout: features on partitions, batch on free axis.
  - r-gate x-projections are pre-accumulated ONCE PER CHUNK directly into
    PSUM: tile [128, 6*512] spans 6 banks, one bank per output m-tile, so
    each bank holds exactly one accumulation group for the whole chunk
    (PSUM allows a group to pause while other banks run groups, but not
    two interleaved groups in one bank). Per-step h-matmuls accumulate on
    top (start=False); sigmoid reads PSUM directly. The z-gate and g
    matmuls use per-step contiguous groups in their own banks (x-part
    matmuls ride in the z burst).
  - Per-chunk cat buffer [p, j(6 k-tiles), slot, b]: x and h regions are
    slot-contiguous so chunk DMAs are dense; single fused r*cat multiply;
    the h-update writes straight into the next step's cat slot; the
    output DMA reads the h regions (no copies).
  - Two cat buffers ping-pong across chunks (loop body = 2 chunks) so
    chunk DMAs overlap compute.
  - Tail trick: zc=1-z and m1=zc*h computed off the critical path; after
    tanh only m2=z*g and h'=m1+m2 remain.
  - Output stored bf16, converted to fp32 on host.
"""

import sys

sys.path.insert(0, "/opt/trn_rl_repo")

import numpy as np
import ml_dtypes

import concourse.bass as bass
import concourse.bacc as bacc
import concourse.mybir as mybir
from concourse.bass import ds
from concourse.tile import TileContext
from concourse.bass_utils import run_bass_kernel_spmd

BF16 = ml_dtypes.bfloat16

L, B, D, LAT = 2048, 64, 256, 512
CAT = D + LAT  # 768
NCORES = 8
BL = B // NCORES  # 8 local batch rows
CH = 64  # timesteps per chunk
FP32 = mybir.dt.float32
BF = mybir.dt.bfloat16
AF = mybir.ActivationFunctionType
ALU = mybir.AluOpType


def build_gru_nc(length=L, ch=CH):
    nc = bacc.Bacc("TRN2", target_bir_lowering=False)

    # ---- DRAM I/O ----
    xt = nc.dram_tensor("xt", [D, length, BL], BF, kind="ExternalInput")
    w_zx = nc.dram_tensor("w_zx", [128, 2 * LAT], BF, kind="ExternalInput")
    w_zh = nc.dram_tensor("w_zh", [128, 4 * LAT], BF, kind="ExternalInput")
    w_rx = nc.dram_tensor("w_rx", [128, 2 * CAT], BF, kind="ExternalInput")
    w_rh = nc.dram_tensor("w_rh", [128, 4 * CAT], BF, kind="ExternalInput")
    w_hx = nc.dram_tensor("w_hx", [128, 2 * LAT], BF, kind="ExternalInput")
    w_hh = nc.dram_tensor("w_hh", [128, 4 * LAT], BF, kind="ExternalInput")
    hs = nc.dram_tensor("hs", [LAT, length, BL], BF, kind="ExternalOutput")

    SL = ch + 1  # slots per chunk buffer

    with TileContext(nc) as tc:
        with (
            tc.tile_pool(name="wpool", bufs=1) as wpool,
            tc.tile_pool(name="sbuf", bufs=1) as sb,
            tc.tile_pool(name="psum", bufs=1, space="PSUM") as pp,
        ):
            # weights resident in SBUF
            s_zx = wpool.tile([128, 2 * LAT], BF, tag="zx")
            s_zh = wpool.tile([128, 4 * LAT], BF, tag="zh")
            s_rx = wpool.tile([128, 2 * CAT], BF, tag="rx")
            s_rh = wpool.tile([128, 4 * CAT], BF, tag="rh")
            s_hx = wpool.tile([128, 2 * LAT], BF, tag="hx")
            s_hh = wpool.tile([128, 4 * LAT], BF, tag="hh")
            for dst, src in [
                (s_zx, w_zx), (s_zh, w_zh), (s_rx, w_rx),
                (s_rh, w_rh), (s_hx, w_hx), (s_hh, w_hh),
            ]:
                nc.sync.dma_start(dst[:, :], src[:, :])

            # cat chunk buffers: [p, j(6), slot(ch+1), b(8)] bf16
            #   j<2: x k-tiles (slots 0..ch-1); j>=2: h k-tiles (slot s holds
            #   h for step s; step s writes h' into slot s+1).
            catc_a = sb.tile([128, 6 * SL * BL], BF, tag="catca")
            catc_b = sb.tile([128, 6 * SL * BL], BF, tag="catcb")
            cav = catc_a[:, :].rearrange("p (j s b) -> p j s b", s=SL, b=BL)
            cbv = catc_b[:, :].rearrange("p (j s b) -> p j s b", s=SL, b=BL)

            # step temporaries
            rb = sb.tile([128, 48], BF, tag="rb")
            rcb = sb.tile([128, 48], BF, tag="rcb")
            zb = sb.tile([128, 32], BF, tag="zb")
            zcb = sb.tile([128, 32], BF, tag="zcb")
            gt = sb.tile([128, 32], BF, tag="gt")
            m1 = sb.tile([128, 32], BF, tag="m1")
            m2 = sb.tile([128, 32], BF, tag="m2")
            rbv = rb[:, :].rearrange("p (j b) -> p j b", b=BL)
            rcbv = rcb[:, :].rearrange("p (j b) -> p j b", b=BL)
            zbv = zb[:, :].rearrange("p (m b) -> p m b", b=BL)
            zcbv = zcb[:, :].rearrange("p (m b) -> p m b", b=BL)
            gtv = gt[:, :].rearrange("p (m b) -> p m b", b=BL)
            m1v = m1[:, :].rearrange("p (m b) -> p m b", b=BL)
            m2v = m2[:, :].rearrange("p (m b) -> p m b", b=BL)

            # r-gate PSUM: 6 banks, one per m-tile; whole chunk per bank.
            pr = pp.tile([128, 6 * ch * BL], FP32, tag="pr")
            prv = pr[:, :].rearrange("p (m s b) -> p m s b", s=ch, b=BL)
            # z / g PSUM: per-step groups, ping-pong by step parity.
            pz = pp.tile([128, 2 * 4 * BL], FP32, tag="pz")
            pzv = pz[:, :].rearrange("p (h m b) -> p h m b", m=4, b=BL)
            pg = pp.tile([128, 2 * 4 * BL], FP32, tag="pg")
            pgv = pg[:, :].rearrange("p (h m b) -> p h m b", m=4, b=BL)

            # initial h = 0 in catc_b's carry slot (chunk A carries from B)
            nc.vector.memset(cbv[:, 2:6, ch, :], 0.0)

            def do_chunk(i0c, cv, pv):
                """One chunk at dram offset i0c using buffer view cv; pv is
                the previous chunk's buffer view (h carry source)."""
                # carry h (prev buffer slot ch -> this buffer slot 0)
                nc.vector.tensor_copy(cv[:, 2:6, 0, :], pv[:, 2:6, ch, :])

                # r x-projections for the whole chunk
                for m in range(6):
                    for k in range(2):
                        nc.tensor.matmul(
                            prv[:, m, :, :],
                            s_rx[:, k * CAT + m * 128 : k * CAT + (m + 1) * 128],
                            cv[:, k, 0:ch, :],
                            start=(k == 0),
                            stop=False,
                            skip_group_check=True,
                        )

                for s in range(ch):
                    half = s % 2

                    # ---- r h-matmuls accumulate on top of x-proj ----
                    for m in range(6):
                        for k in range(4):
                            nc.tensor.matmul(
                                prv[:, m, s, :],
                                s_rh[:, k * CAT + m * 128 : k * CAT + (m + 1) * 128],
                                cv[:, 2 + k, s, :],
                                start=False,
                                stop=(k == 3),
                                skip_group_check=True,
                            )
                    # ---- z: full per-step groups (x + h contraction) ----
                    for m in range(4):
                        for k in range(2):
                            nc.tensor.matmul(
                                pzv[:, half, m, :],
                                s_zx[:, k * LAT + m * 128 : k * LAT + (m + 1) * 128],
                                cv[:, k, s, :],
                                start=(k == 0),
                                stop=False,
                            )
                        for k in range(4):
                            nc.tensor.matmul(
                                pzv[:, half, m, :],
                                s_zh[:, k * LAT + m * 128 : k * LAT + (m + 1) * 128],
                                cv[:, 2 + k, s, :],
                                start=False,
                                stop=(k == 3),
                            )

                    # ---- gates ----
                    nc.scalar.activation(rbv, prv[:, :, s, :], AF.Sigmoid)
                    nc.scalar.activation(zbv, pzv[:, half, :, :], AF.Sigmoid)
                    # rc = r * cat  (critical); zc = 1 - z; m1 = zc * h
                    nc.vector.tensor_mul(rcbv, rbv, cv[:, :, s, :])
                    nc.vector.tensor_scalar(
                        zcb[:, :], zb[:, :], -1.0, 1.0, ALU.mult, ALU.add
                    )
                    nc.vector.tensor_mul(m1v, zcbv, cv[:, 2:6, s, :])

                    # ---- g matmuls ----
                    for m in range(4):
                        for j in range(6):
                            if j < 2:
                                w = s_hx[:, j * LAT + m * 128 : j * LAT + (m + 1) * 128]
                            else:
                                w = s_hh[
                                    :, (j - 2) * LAT + m * 128 : (j - 2) * LAT + (m + 1) * 128
                                ]
                            nc.tensor.matmul(
                                pgv[:, half, m, :],
                                w,
                                rcbv[:, j, :],
                                start=(j == 0),
                                stop=(j == 5),
                            )

                    # ---- tail: h' = m1 + z*g ----
                    nc.scalar.activation(gtv, pgv[:, half, :, :], AF.Tanh)
                    nc.vector.tensor_mul(m2v, zbv, gtv)
                    nc.vector.tensor_add(cv[:, 2:6, s + 1, :], m1v, m2v)

                # ---- store chunk output (h slots 1..ch) ----
                for k in range(4):
                    nc.sync.dma_start(
                        hs[128 * k : 128 * (k + 1), ds(i0c, ch), :],
                        cv[:, 2 + k, 1 : ch + 1, :],
                    )

            with tc.For_i(
                0, length, 2 * ch,
                staggered_reset=True,
                hint_engines=(
                    mybir.EngineType.PE,
                    mybir.EngineType.DVE,
                    mybir.EngineType.Activation,
                    mybir.EngineType.SP,
                ),
            ) as i0:
                # prefetch x for both chunks of this iteration
                for k in range(2):
                    nc.sync.dma_start(
                        cav[:, k, 0:ch, :],
                        xt[128 * k : 128 * (k + 1), ds(i0, ch), :],
                    )
                for k in range(2):
                    nc.sync.dma_start(
                        cbv[:, k, 0:ch, :],
                        xt[128 * k : 128 * (k + 1), ds(i0 + ch, ch), :],
                    )
                do_chunk(i0, cav, cbv)
                do_chunk(i0 + ch, cbv, cav)
    nc.compile()
    return nc


def _pack_lhsT(w):
    """[K, M] lhsT -> [128, (K//128)*M] packed, col = ktile*M + m."""
    K, M = w.shape
    return (
        w.reshape(K // 128, 128, M).transpose(1, 0, 2).reshape(128, -1)
    )


def prep_weights(Wz, Wr, Wh):
    out = {}
    for name, W, xd in [("z", Wz, LAT), ("r", Wr, CAT), ("h", Wh, LAT)]:
        lhsT_x = _pack_lhsT(np.ascontiguousarray(W[:, :D].T))  # [256, M]
        lhsT_h = _pack_lhsT(np.ascontiguousarray(W[:, D:].T))  # [512, M]
        out[f"w_{name}x"] = lhsT_x.astype(BF16)
        out[f"w_{name}h"] = lhsT_h.astype(BF16)
    return out


_nc_cache = {}


def kernel(x, Wz, Wr, Wh, _nc_cache=_nc_cache):
    x = np.asarray(x, np.float32)
    Wz = np.asarray(Wz, np.float32)
    Wr = np.asarray(Wr, np.float32)
    Wh = np.asarray(Wh, np.float32)

    key = "nc"
    if key not in _nc_cache:
        _nc_cache[key] = build_gru_nc()
    nc = _nc_cache[key]

    wmap = prep_weights(Wz, Wr, Wh)
    xt_all = np.ascontiguousarray(x.transpose(2, 0, 1)).astype(BF16)  # [D, L, B]

    in_maps = []
    for c in range(NCORES):
        m = dict(wmap)
        m["xt"] = np.ascontiguousarray(xt_all[:, :, c * BL : (c + 1) * BL])
        in_maps.append(m)

    res = run_bass_kernel_spmd(nc, in_maps, core_ids=list(range(NCORES)))
    outs = []
    for c in range(NCORES):
        hsT = np.asarray(res.results[c]["hs"]).astype(np.float32)  # [LAT, L, BL]
        outs.append(hsT.transpose(1, 2, 0))  # [L, BL, LAT]
    return np.concatenate(outs, axis=1)  # [L, B, LAT]
